# revision 2
# baseline (speedup 1.0000x reference)
"""DePatchEfficient Trainium2 kernel, v2: PE/DVE/Pool three-way split.

Reconstructs a (B, U, V, S, T, C) volume from overlapping 4D patches by
scatter-add + overlap-count division (overlap-add, 16 polyphase terms).

v1 (baseline, 73us) was paced by the DMA stream: terms landed mostly as
fp16 (SWDGE cast in flight, 2B/elem SBUF-landed) because the DVE needs
fp16 operands for its fast 2x adds. v2 lands EVERYTHING as raw int8
(1B/elem, 8.47MB/core: the hard floor) and splits the adds across three
engines so none exceeds the ~31us DMA stream time:

- PE (tensor): owns the rv=0 half of the output volume. Act casts the
  int8 slabs to fp16 staging; the PE accumulates them into PSUM with
  0/1 "router" stationary matrices (lhsT[r, p] = 1 routes slab row r to
  acc partition p), which absorb the (e, f) partition shifts and the
  row clipping. PSUM fp32 accumulate-on-write (start=True clears the
  bank; later matmuls add where written, overwrite where not) gives
  scatter-accumulate with free zero-init.
- DVE (vector): owns rv=1, c in {0,1} (+ the qs>=5 slice of rv=1,c=2),
  with the v1 structure: int8 adds (1x mode) into an SBUF fp16 acc.
- Pool (gpsimd): owns rv=1, c=2, qs<5, plus casts 4 of the 16 PE slabs
  (Act casts the other 12 and evacuates PSUM->SBUF fp16 at the end).

All sums are integer-valued and bounded by 16*127 = 2032 < 2048, exact
in fp16/fp32; the only error is the host-side int8 quantization
(~5.7e-3 rel, gate is 2e-2). Scale and 1/overlap-count fold into the
host-side assembly.

Sharding: 8 cores = (batch b2) x (s-half) x (t-half); high halves are
axis-flipped host-side so all cores run an identical program (v1 trick).

Per-core layouts:
- PE psum acc [128, 3072] fp32 (6 banks): partition p = qs*16+qt*2+ru,
  free = mu*384 + mv*48 + ws*12 + wt*3 + c   (v = 2*mv + 0)
- PE slab [R, 2352] int8, R = (8-e)(8-f)*2: row r = is*(8-f)*2+it*2+ru,
  free = iu*336 + iv*48 + ws*12 + wt*3 + c
- DVE acc [128, 2432] fp16: partition = qt'*16 + ws*4 + wt (qt'=7-qt);
  free[0:2048] = mu*256 + mv*32 + qs*4 + ru*2 + c (c in {0,1});
  free[2048:] = mu*48 + mv*6 + (qs-5)*2 + ru (c=2, qs in [5,8))
- Pool acc [128, 640] fp16: partition as DVE; free = mu*80 + mv*10 +
  qs*2 + ru (c=2, qs in [0,5))
"""

import os
import sys

import numpy as np

for _p in ("/opt/trn_rl_repo",):
    if os.path.isdir(_p) and _p not in sys.path:
        sys.path.insert(0, _p)

B, U, V, S, T, C = 2, 16, 16, 64, 64, 3
NS, NT, NU, NV = 15, 15, 7, 7

GROUPS = [(1, 1), (1, 0), (0, 1), (0, 0)]      # (e, f) emission order
AB = [(0, 0), (0, 1), (1, 0), (1, 1)]          # (a, b) within a group
QPOOL = 5                                      # pool owns g5 qs < QPOOL
# PE slabs cast by Pool instead of Act (one per group, the last (a,b)):
POOL_CAST = {(e, f, 1, 1) for (e, f) in GROUPS}

PE_F = 3072            # PE psum free size (fp32)
DVE_F = 2432           # DVE acc free size (fp16)
POOL_F = 640           # Pool acc free size (fp16)


def _pe_rows(e, f):
    return (8 - e) * (8 - f) * 2


def _dve_rows(f):
    return (8 - f) * 16


def _g34_len(e):
    return 49 * (8 - e) * 4


G5HI_LEN = 49 * 3 * 2  # 294


def _pool_len(e):
    return 49 * (5 - e) * 2


def _lhst_mats():
    """4 router matrices [128, 128] fp16, one per (e, f) group, plus a
    zero matrix used by the bank-zeroing matmuls."""
    mats = np.zeros((5, 128, 128), np.float16)
    for gi, (e, f) in enumerate(GROUPS):
        for is_ in range(8 - e):
            for it in range(8 - f):
                for ru in range(2):
                    r = is_ * (8 - f) * 2 + it * 2 + ru
                    p = (is_ + e) * 16 + (it + f) * 2 + ru
                    mats[gi, r, p] = 1.0
    return mats


_LHST = _lhst_mats()


def _matmul_pieces():
    """Emission-ordered list of (term_idx, piece) where piece =
    (psum_off, length, rhs_col0). Split per iu at 512 (bank) bounds."""
    out = []
    for ti, (e, f) in enumerate(GROUPS):
        for ai, (a, b) in enumerate(AB):
            t = ti * 4 + ai
            for iu in range(7):
                o = (iu + a) * 384 + b * 48
                end = o + 336
                cuts = [o] + [k * 512 for k in range(1, 7) if o < k * 512 < end] + [end]
                for s0, s1 in zip(cuts[:-1], cuts[1:]):
                    out.append((t, s0, s1 - s0, iu * 336 + (s0 - o)))
    return out


_PIECES = _matmul_pieces()
# first/last piece index per psum bank (for start/stop flags)
_BANK_FIRST, _BANK_LAST = {}, {}
for _i, (_t, _o, _l, _c) in enumerate(_PIECES):
    _bk = _o // 512
    _BANK_FIRST.setdefault(_bk, _i)
    _BANK_LAST[_bk] = _i


def _shard(x):
    """Full input (B, 11025, 4, 4, 8, 8, 3) -> per-core in_maps + scales.

    Per core, builds the packed int8 DMA chunk buffers:
    - pe_c{k}: 8 chunks of 2 PE slabs each (group order)
    - dve_c{k}: 8 chunks of 2 DVE term buffers ([g34 | g5hi] per term)
    - pool_c{0,1}: f=1 and f=0 class fat buffers (group order inside)
    - lhst: the 4 router matrices
    """
    x9 = np.ascontiguousarray(x).reshape(B, NS, NT, NU, NV, 4, 4, 8, 8, C)
    in_maps, scales = [], []
    for core in range(8):
        b, sh, th = core // 4, (core // 2) % 2, core % 2
        xc = x9[b, 7 * sh : 7 * sh + 8, 7 * th : 7 * th + 8]
        if sh:
            xc = xc[::-1, :, :, :, :, :, ::-1]
        if th:
            xc = xc[:, ::-1, :, :, :, :, :, ::-1]
        s = float(np.abs(xc).max()) / 127.0
        xq = np.clip(np.rint(xc * (1.0 / s)), -127, 127).astype(np.int8)
        # (is, it, iu, iv, a, ru, b, rv, e, ws, f, wt, c)
        xq = xq.reshape(8, 8, NU, NV, 2, 2, 2, 2, 2, 4, 2, 4, C)
        scales.append(s)
        m = {"lhst": _LHST.transpose(1, 0, 2).reshape(128, 640).copy()}

        pe_slabs, dve_bufs = [], []
        pool_slabs = {0: [], 1: []}
        for (e, f) in GROUPS:
            for (a, bb) in AB:
                # PE slab: rv=0, all c -> (is, it, ru | iu, iv, ws, wt, c)
                sl = xq[: 8 - e, : 8 - f, :, :, a, :, bb, 0, e, :, f, :, :]
                sl = sl.transpose(0, 1, 4, 2, 3, 5, 6, 7)
                pe_slabs.append(
                    np.ascontiguousarray(sl.reshape(_pe_rows(e, f), 2352))
                )
                # DVE g34: rv=1, c<2 -> (it', ws, wt | iu, iv, is, ru, c)
                g34 = xq[: 8 - e, : 8 - f, :, :, a, :, bb, 1, e, :, f, :, :2]
                g34 = g34[:, ::-1].transpose(1, 5, 6, 2, 3, 0, 4, 7)
                g34 = g34.reshape(_dve_rows(f), _g34_len(e))
                # DVE g5hi: rv=1, c=2, is in [5-e, 8-e)
                g5 = xq[5 - e : 8 - e, : 8 - f, :, :, a, :, bb, 1, e, :, f, :, 2]
                g5 = g5[:, ::-1].transpose(1, 5, 6, 2, 3, 0, 4)
                g5 = g5.reshape(_dve_rows(f), G5HI_LEN)
                dve_bufs.append(np.concatenate([g34, g5], axis=1))
                # Pool: rv=1, c=2, is in [0, 5-e)
                pl = xq[: 5 - e, : 8 - f, :, :, a, :, bb, 1, e, :, f, :, 2]
                pl = pl[:, ::-1].transpose(1, 5, 6, 2, 3, 0, 4)
                pool_slabs[f].append(pl.reshape(_dve_rows(f), _pool_len(e)))

        for k in range(8):
            m[f"pe_c{k}"] = np.ascontiguousarray(
                np.concatenate(pe_slabs[2 * k : 2 * k + 2], axis=1)
            )
            m[f"dve_c{k}"] = np.ascontiguousarray(
                np.concatenate(dve_bufs[2 * k : 2 * k + 2], axis=1)
            )
        m["pool_c0"] = np.ascontiguousarray(np.concatenate(pool_slabs[1], axis=1))
        m["pool_c1"] = np.ascontiguousarray(np.concatenate(pool_slabs[0], axis=1))
        in_maps.append(m)
    return in_maps, scales


def _count_map():
    cu = np.array([1, 2, 2, 2, 2, 2, 2, 1], np.float32)[np.arange(U) // 2]
    cs = np.array([1, 2, 2, 2, 2, 2, 2, 2], np.float32)[np.arange(32) // 4]
    cnt = (cu[:, None, None, None] * cu[None, :, None, None]
           * cs[None, None, :, None] * cs[None, None, None, :])
    return cnt[..., None]  # (U, V, 32, 32, 1)


_CNT = _count_map()


def _assemble(core_outs, scales):
    """Per-core {out_pe, out_dve, out_pool} -> full (B,U,V,S,T,C) fp32."""
    full = np.empty((B, U, V, S, T, C), np.float32)
    mu_ = np.arange(8)
    for core in range(8):
        b, sh, th = core // 4, (core // 2) % 2, core % 2
        vol = np.zeros((U, V, 32, 32, C), np.float32)
        # PE region: v even
        pe = np.asarray(core_outs[core]["out_pe"], np.float32)
        pe = pe.reshape(8, 8, 2, 8, 8, 4, 4, 3)  # (qs, qt, ru, mu, mv, ws, wt, c)
        # -> (mu, ru, mv, qs, ws, qt, wt, c) -> (u, v_e, s, t, c)
        pe = pe.transpose(3, 2, 4, 0, 5, 1, 6, 7).reshape(U, 8, 32, 32, C)
        vol[:, 0::2] = pe
        # DVE region: v odd, c in {0,1}
        dv = np.asarray(core_outs[core]["out_dve"], np.float32)
        g34 = dv[:, :2048].reshape(8, 4, 4, 8, 8, 8, 2, 2)
        # (qt', ws, wt, mu, mv, qs, ru, c) ; qt = 7 - qt'
        g34 = g34[::-1].transpose(3, 6, 4, 5, 1, 0, 2, 7)
        # -> (mu, ru, mv, qs, ws, qt, wt, c) -> (u, v_o, s, t, c2)
        vol[:, 1::2, :, :, :2] = g34.reshape(U, 8, 32, 32, 2)
        g5 = dv[:, 2048:].reshape(8, 4, 4, 8, 8, 3, 2)
        g5 = g5[::-1].transpose(3, 6, 4, 5, 1, 0, 2)  # (mu, ru, mv, qs3, ws, qt, wt)
        vol[:, 1::2, 20:, :, 2] = g5.reshape(U, 8, 12, 32)
        # Pool region: v odd, c=2, qs<5
        pl = np.asarray(core_outs[core]["out_pool"], np.float32)
        pl = pl.reshape(8, 4, 4, 8, 8, 5, 2)
        pl = pl[::-1].transpose(3, 6, 4, 5, 1, 0, 2)
        vol[:, 1::2, :20, :, 2] = pl.reshape(U, 8, 20, 32)

        vol *= scales[core] / _CNT
        if sh:
            vol = vol[:, :, ::-1]
        if th:
            vol = vol[:, :, :, ::-1]
        full[b, :, :, 32 * sh : 32 * sh + 32, 32 * th : 32 * th + 32, :] = vol
    return full


def build_nc():
    import concourse.bacc as bacc
    import concourse.mybir as mybir
    from concourse.tile import TileContext

    nc = bacc.Bacc("TRN2", target_bir_lowering=False, debug=False)

    lhst_d = nc.dram_tensor("lhst", [128, 640], mybir.dt.float16, kind="ExternalInput")
    pe_d, dve_d = [], []
    for k in range(8):
        e, f = GROUPS[k // 2]
        pe_d.append(nc.dram_tensor(
            f"pe_c{k}", [_pe_rows(e, f), 2 * 2352], mybir.dt.int8,
            kind="ExternalInput"))
        dve_d.append(nc.dram_tensor(
            f"dve_c{k}", [_dve_rows(f), 2 * (_g34_len(e) + G5HI_LEN)],
            mybir.dt.int8, kind="ExternalInput"))
    pool_d = [
        nc.dram_tensor("pool_c0", [112, 4 * 392 + 4 * 490], mybir.dt.int8,
                       kind="ExternalInput"),
        nc.dram_tensor("pool_c1", [128, 4 * 392 + 4 * 490], mybir.dt.int8,
                       kind="ExternalInput"),
    ]
    out_pe = nc.dram_tensor("out_pe", [128, PE_F], mybir.dt.float16,
                            kind="ExternalOutput")
    out_dve = nc.dram_tensor("out_dve", [128, DVE_F], mybir.dt.float16,
                             kind="ExternalOutput")
    out_pool = nc.dram_tensor("out_pool", [128, POOL_F], mybir.dt.float16,
                              kind="ExternalOutput")

    with (
        TileContext(nc) as tc,
        tc.tile_pool(name="inp", bufs=1) as inp,
        tc.tile_pool(name="accp", bufs=1) as accp,
        tc.tile_pool(name="stgp", bufs=4) as stgp,
        tc.tile_pool(name="psp", bufs=1, space="PSUM") as psp,
    ):
        lhst = inp.tile([128, 640], mybir.dt.float16)
        pe8 = [inp.tile([_pe_rows(*GROUPS[k // 2]), 2 * 2352], mybir.dt.int8,
                        name=f"pe8_{k}") for k in range(8)]
        dve8 = [inp.tile(
            [_dve_rows(GROUPS[k // 2][1]),
             2 * (_g34_len(GROUPS[k // 2][0]) + G5HI_LEN)],
            mybir.dt.int8, name=f"dve8_{k}") for k in range(8)]
        pool8 = [inp.tile([112, 3528], mybir.dt.int8, name="pool8_0"),
                 inp.tile([128, 3528], mybir.dt.int8, name="pool8_1")]
        acc_d = accp.tile([128, DVE_F], mybir.dt.float16)
        acc_p = accp.tile([128, POOL_F], mybir.dt.float16)
        evac = accp.tile([128, PE_F], mybir.dt.float16)
        psum = psp.tile([128, PE_F], mybir.dt.float32)

        # --- DMAs (sync HWDGE queue), interleaved for pipelining ---
        nc.sync.dma_start(out=lhst[:, :], in_=lhst_d.ap())
        for k in range(8):
            nc.sync.dma_start(out=dve8[k][:, :], in_=dve_d[k].ap())
            nc.sync.dma_start(out=pe8[k][:, :], in_=pe_d[k].ap())
            if k == 1:
                nc.sync.dma_start(out=pool8[0][:, :], in_=pool_d[0].ap())
            if k == 3:
                nc.sync.dma_start(out=pool8[1][:, :], in_=pool_d[1].ap())

        # --- accumulator zeroing (during first DMAs) ---
        nc.vector.memset(acc_d[:, :].bitcast(mybir.dt.uint32), 0)
        nc.gpsimd.memset(acc_p[:, :].bitcast(mybir.dt.uint32), 0)

        acc4 = acc_d[:, :2048].rearrange("p (mu mv q) -> p mu mv q",
                                         mu=8, mv=8, q=32)
        acc5 = acc_d[:, 2048:].rearrange("p (mu q) -> p mu q", mu=8, q=48)
        accq = acc_p[:, :].rearrange("p (mu mv q) -> p mu mv q",
                                     mu=8, mv=8, q=10)

        st16 = [stgp.tile([128, 2352], mybir.dt.float16, tag="st16",
                          name=f"st16_{i}") for i in range(16)]

        # Open every psum bank with a zeroing matmul (zero lhsT): sets the
        # whole bank's has_written bits so all real matmuls accumulate.
        for k in range(6):
            nc.tensor.matmul(
                out=psum[:, 512 * k : 512 * k + 512],
                lhsT=lhst[:, 512:640],
                rhs=lhst[:, 0:512],
                start=True,
                stop=False,
            )

        # --- per-term ops, group-ordered ---
        pool_off = {1: 0, 0: 0}
        piece_idx = 0
        for ti, (e, f) in enumerate(GROUPS):
            gR = _pe_rows(e, f)
            dR = _dve_rows(f)
            lh = lhst[0:gR, 128 * ti : 128 * ti + 128]
            for ai, (a, bb) in enumerate(AB):
                t = ti * 4 + ai
                # cast the PE slab
                src = pe8[t // 2][0:gR, (t % 2) * 2352 : (t % 2) * 2352 + 2352]
                dst = st16[t][0:gR, :]
                if (e, f, a, bb) in POOL_CAST:
                    nc.gpsimd.tensor_copy(out=dst, in_=src)
                else:
                    nc.scalar.copy(out=dst, in_=src)
                # PE matmuls for this term
                while piece_idx < len(_PIECES) and _PIECES[piece_idx][0] == t:
                    _, po, plen, col0 = _PIECES[piece_idx]
                    nc.tensor.matmul(
                        out=psum[:, po : po + plen],
                        lhsT=lh,
                        rhs=st16[t][0:gR, col0 : col0 + plen],
                        start=False,
                        stop=_BANK_LAST[po // 512] == piece_idx,
                    )
                    piece_idx += 1
                # DVE adds
                tl = _g34_len(e) + G5HI_LEN
                base = (t % 2) * tl
                g34 = dve8[t // 2][0:dR, base : base + _g34_len(e)].rearrange(
                    "p (iu iv q) -> p iu iv q", iu=7, iv=7, q=(8 - e) * 4)
                ov = acc4[0:dR, a : a + 7, bb : bb + 7, 4 * e : 32]
                nc.vector.tensor_add(out=ov, in0=ov, in1=g34)
                g5 = dve8[t // 2][0:dR,
                                  base + _g34_len(e) : base + tl].rearrange(
                    "p (iu q) -> p iu q", iu=7, q=42)
                ov5 = acc5[0:dR, a : a + 7, 6 * bb : 6 * bb + 42]
                nc.vector.tensor_add(out=ov5, in0=ov5, in1=g5)
                # Pool add
                pf = _pool_len(e)
                pti = 0 if f == 1 else 1
                pb = pool_off[f]
                pool_off[f] += pf
                pv = pool8[pti][0:dR, pb : pb + pf].rearrange(
                    "p (iu iv q) -> p iu iv q", iu=7, iv=7, q=(5 - e) * 2)
                ovp = accq[0:dR, a : a + 7, bb : bb + 7, 2 * e : 10]
                nc.gpsimd.tensor_add(out=ovp, in0=ovp, in1=pv)

        # --- tail: evacuate PSUM, write outputs ---
        nc.scalar.copy(out=evac[:, :], in_=psum[:, :])
        nc.sync.dma_start(out=out_dve.ap(), in_=acc_d[:, :])
        nc.sync.dma_start(out=out_pool.ap(), in_=acc_p[:, :])
        nc.scalar.dma_start(out=out_pe.ap(), in_=evac[:, :])
    nc.compile()
    return nc


def kernel(x):
    x = np.ascontiguousarray(np.asarray(x), dtype=np.float32)
    in_maps, scales = _shard(x)
    nc = build_nc()
    from concourse.bass_utils import run_bass_kernel_spmd

    res = run_bass_kernel_spmd(nc, in_maps, core_ids=list(range(8)))
    return _assemble(res.results, scales)


# revision 3
# speedup vs baseline: 1.5308x; 1.5308x over previous
"""DePatchEfficient Trainium2 kernel, v3: PE-major split with on-chip widening.

Overlap-add of 16 polyphase terms. All measured-rate driven (v2 post-mortem):
- DVE int8-source adds run ~1x + 470ns/op (not 2x); Pool tensor ops are
  ~0.2 efficiency (useless); Act casts are 1x + 580ns/op; DVE int8->fp16
  copies run ~2x + 470ns; PE self-loading fp16 matmuls run warm at
  ~N/2.4GHz + 22ns with FWL.

Split (output cells, by (rv, c) inner combo):
- PE owns 4 of 6 combos (rv=0 all c, plus rv=1 c=0): 2/3 of the volume,
  full 8-bank PSUM fp32 acc. Slabs widened to fp16 staging by: Act (bulk,
  one op per group's act-run), DVE tensor_copy (2x), or SWDGE cast-DMA
  (lands fp16 directly). Router 0/1 lhsT absorbs (e,f) shifts + clipping.
  mu stride = 512 = one PSUM bank: every (term, iu) matmul is one
  448-col bank-aligned piece - no bank splitting.
- DVE owns 2 combos (rv=1, c in {1,2}): direct int8 adds into SBUF fp16
  acc, v1-style partition (qt', ws, wt) layout.
- Pool: DMA issue only (SWDGE queue for PE slabs).

PSUM banks are opened by 8 zeroing matmuls (zero lhsT): sets has_written
for every cell so all real matmuls accumulate; no memset needed.

Layouts per core (all cores identical after host-side s/t flips):
- PE psum acc 8 x [128, 512] fp32: partition p = qs*16+qt*2+ru,
  flat free = mu*512 + mv*64 + ws*16 + wt*4 + cmb, cmb in
  {(rv,c)} = [(0,0),(0,1),(0,2),(1,0)]
- PE slab [R, 3136] (R = (8-e)(8-f)*2): row = is*(8-f)*2 + it*2 + ru,
  free = iu*448 + iv*64 + ws*16 + wt*4 + cmb
- DVE acc [128, 2048] fp16: partition = qt'*16 + ws*4 + wt (qt' = 7-qt),
  free = mu*256 + mv*32 + qs*4 + ru*2 + (c-1)
- DVE slab [(8-f)*16, (8-e)*196]: row = qt'*16+ws*4+wt,
  free = iu*(8-e)*28 + iv*(8-e)*4 + is*4 + ru*2 + (c-1)
"""

import os
import sys

import numpy as np

for _p in ("/opt/trn_rl_repo",):
    if os.path.isdir(_p) and _p not in sys.path:
        sys.path.insert(0, _p)

B, U, V, S, T, C = 2, 16, 16, 64, 64, 3
NS, NT, NU, NV = 15, 15, 7, 7

GROUPS = [(1, 1), (1, 0), (0, 1), (0, 0)]      # (e, f) order
AB = [(0, 0), (0, 1), (1, 0), (1, 1)]          # (a, b) within a group
# widening route per term index 0..15: "act" (group fat op), "dve"
# (tensor_copy), "dma" (SWDGE cast-DMA lands fp16 directly)
WIDEN = {3: "dma", 7: "dve", 11: "dma", 14: "dve", 15: "dve"}

PE_SLAB_F = 3136       # (iu7, iv7, ws4, wt4, cmb4)
PE_F = 4096            # psum free: (mu8, mv8, ws4, wt4, cmb4)
DVE_F = 2048


def _pe_rows(e, f):
    return (8 - e) * (8 - f) * 2


def _dve_rows(f):
    return (8 - f) * 16


def _dve_len(e):
    return 49 * (8 - e) * 4


def _acts_in(g):
    """Term indices in group g widened by Act (contiguous run, packed)."""
    return [4 * g + i for i in range(4) if WIDEN.get(4 * g + i) is None]


def _lhst_mats():
    """4 router matrices + 1 zero matrix, [128, 128] fp16 each."""
    mats = np.zeros((5, 128, 128), np.float16)
    for gi, (e, f) in enumerate(GROUPS):
        for is_ in range(8 - e):
            for it in range(8 - f):
                for ru in range(2):
                    r = is_ * (8 - f) * 2 + it * 2 + ru
                    p = (is_ + e) * 16 + (it + f) * 2 + ru
                    mats[gi, r, p] = 1.0
    return mats


_LHST = _lhst_mats()

# emission-ordered matmul pieces (term, iu, bank) and per-bank last index
_PIECES = []
for _g in range(4):
    for _ai, (_a, _b) in enumerate(AB):
        for _iu in range(7):
            _PIECES.append((4 * _g + _ai, _iu, _iu + _a, _b))
_BANK_LAST = {}
for _i, (_t, _iu, _m, _b) in enumerate(_PIECES):
    _BANK_LAST[_m] = _i


def _shard(x):
    """Full input -> per-core in_maps + scales.

    Per core buffers:
    - lhst [128, 640] fp16
    - pe_act{g} [R, len(_acts_in(g))*3136] int8  (group-major packed)
    - pe_x{t} [R, 3136] int8 (dve-widened) or fp16 (dma-widened)
    - dve_c{k} [R', 2*dve_len] int8, k = 0..7 (2 terms each)
    """
    x9 = np.ascontiguousarray(x).reshape(B, NS, NT, NU, NV, 4, 4, 8, 8, C)
    in_maps, scales = [], []
    for core in range(8):
        b, sh, th = core // 4, (core // 2) % 2, core % 2
        xc = x9[b, 7 * sh : 7 * sh + 8, 7 * th : 7 * th + 8]
        if sh:
            xc = xc[::-1, :, :, :, :, :, ::-1]
        if th:
            xc = xc[:, ::-1, :, :, :, :, :, ::-1]
        s = float(np.abs(xc).max()) / 127.0
        xq = np.clip(np.rint(xc * (1.0 / s)), -127, 127).astype(np.int8)
        # (is, it, iu, iv, a, ru, b, rv, e, ws, f, wt, c)
        xq = xq.reshape(8, 8, NU, NV, 2, 2, 2, 2, 2, 4, 2, 4, C)
        scales.append(s)
        m = {"lhst": _LHST.transpose(1, 0, 2).reshape(128, 640).copy()}

        pe_slabs, dve_bufs = [], []
        for (e, f) in GROUPS:
            for (a, bb) in AB:
                # PE slab: (is, it, ru | iu, iv, ws, wt, cmb4)
                # cmb: (rv=0, c=0..2) then (rv=1, c=0)
                r0 = xq[: 8 - e, : 8 - f, :, :, a, :, bb, 0, e, :, f, :, :]
                r1 = xq[: 8 - e, : 8 - f, :, :, a, :, bb, 1, e, :, f, :, 0:1]
                sl = np.concatenate([r0, r1], axis=7)  # (...ws, wt, cmb4)
                sl = sl.transpose(0, 1, 4, 2, 3, 5, 6, 7)
                pe_slabs.append(
                    np.ascontiguousarray(sl.reshape(_pe_rows(e, f), PE_SLAB_F)))
                # DVE slab: rv=1, c in {1,2} -> (it', ws, wt | iu, iv, is, ru, c)
                g2 = xq[: 8 - e, : 8 - f, :, :, a, :, bb, 1, e, :, f, :, 1:]
                g2 = g2[:, ::-1].transpose(1, 5, 6, 2, 3, 0, 4, 7)
                dve_bufs.append(
                    np.ascontiguousarray(g2.reshape(_dve_rows(f), _dve_len(e))))

        for g in range(4):
            acts = _acts_in(g)
            m[f"pe_act{g}"] = np.ascontiguousarray(
                np.concatenate([pe_slabs[t] for t in acts], axis=1))
        for t, route in WIDEN.items():
            sl = pe_slabs[t]
            m[f"pe_x{t}"] = sl if route == "dve" else sl.astype(np.float16)
        for k in range(8):
            m[f"dve_c{k}"] = np.ascontiguousarray(
                np.concatenate(dve_bufs[2 * k : 2 * k + 2], axis=1))
        in_maps.append(m)
    return in_maps, scales


def _count_map():
    cu = np.array([1, 2, 2, 2, 2, 2, 2, 1], np.float32)[np.arange(U) // 2]
    cs = np.array([1, 2, 2, 2, 2, 2, 2, 2], np.float32)[np.arange(32) // 4]
    cnt = (cu[:, None, None, None] * cu[None, :, None, None]
           * cs[None, None, :, None] * cs[None, None, None, :])
    return cnt[..., None]  # (U, V, 32, 32, 1)


_CNT = _count_map()


def _assemble(core_outs, scales):
    """Per-core {out_pe, out_dve} -> full (B, U, V, S, T, C) fp32."""
    full = np.empty((B, U, V, S, T, C), np.float32)
    for core in range(8):
        b, sh, th = core // 4, (core // 2) % 2, core % 2
        vol = np.zeros((U, V, 32, 32, C), np.float32)
        pe = np.asarray(core_outs[core]["out_pe"], np.float32)
        # (qs, qt, ru | mu, mv, ws, wt, cmb)
        pe = pe.reshape(8, 8, 2, 8, 8, 4, 4, 4)
        # -> (mu, ru, mv, qs, ws, qt, wt, cmb)
        pe = pe.transpose(3, 2, 4, 0, 5, 1, 6, 7).reshape(U, 8, 32, 32, 4)
        vol[:, 0::2, :, :, :] = pe[..., 0:3]        # rv=0, c=0..2
        vol[:, 1::2, :, :, 0] = pe[..., 3]          # rv=1, c=0
        dv = np.asarray(core_outs[core]["out_dve"], np.float32)
        # (qt', ws, wt | mu, mv, qs, ru, c2)
        dv = dv.reshape(8, 4, 4, 8, 8, 8, 2, 2)
        dv = dv[::-1].transpose(3, 6, 4, 5, 1, 0, 2, 7)
        vol[:, 1::2, :, :, 1:] = dv.reshape(U, 8, 32, 32, 2)

        vol *= scales[core] / _CNT
        if sh:
            vol = vol[:, :, ::-1]
        if th:
            vol = vol[:, :, :, ::-1]
        full[b, :, :, 32 * sh : 32 * sh + 32, 32 * th : 32 * th + 32, :] = vol
    return full


def build_nc():
    import concourse.bacc as bacc
    import concourse.mybir as mybir
    from concourse.tile import TileContext

    nc = bacc.Bacc("TRN2", target_bir_lowering=False, debug=False)

    lhst_d = nc.dram_tensor("lhst", [128, 640], mybir.dt.float16,
                            kind="ExternalInput")
    pe_act_d, pe_x_d, dve_d = {}, {}, []
    for g, (e, f) in enumerate(GROUPS):
        pe_act_d[g] = nc.dram_tensor(
            f"pe_act{g}", [_pe_rows(e, f), len(_acts_in(g)) * PE_SLAB_F],
            mybir.dt.int8, kind="ExternalInput")
    for t, route in WIDEN.items():
        e, f = GROUPS[t // 4]
        pe_x_d[t] = nc.dram_tensor(
            f"pe_x{t}", [_pe_rows(e, f), PE_SLAB_F],
            mybir.dt.int8 if route == "dve" else mybir.dt.float16,
            kind="ExternalInput")
    for k in range(8):
        e, f = GROUPS[k // 2]
        dve_d.append(nc.dram_tensor(
            f"dve_c{k}", [_dve_rows(f), 2 * _dve_len(e)], mybir.dt.int8,
            kind="ExternalInput"))
    out_pe = nc.dram_tensor("out_pe", [128, PE_F], mybir.dt.float16,
                            kind="ExternalOutput")
    out_dve = nc.dram_tensor("out_dve", [128, DVE_F], mybir.dt.float16,
                             kind="ExternalOutput")

    with (
        TileContext(nc) as tc,
        tc.tile_pool(name="inp", bufs=1) as inp,
        tc.tile_pool(name="stgp", bufs=2) as stgp,
        tc.tile_pool(name="psp", bufs=1, space="PSUM") as psp,
    ):
        lhst = inp.tile([128, 640], mybir.dt.float16)
        pe8 = {g: inp.tile([_pe_rows(*GROUPS[g]), len(_acts_in(g)) * PE_SLAB_F],
                           mybir.dt.int8, name=f"pe8_{g}") for g in range(4)}
        x16 = {t: inp.tile([_pe_rows(*GROUPS[t // 4]), PE_SLAB_F],
                           mybir.dt.float16, name=f"x16_{t}")
               for t in WIDEN}
        x8 = {t: inp.tile([_pe_rows(*GROUPS[t // 4]), PE_SLAB_F],
                          mybir.dt.int8, name=f"x8_{t}")
              for t, route in WIDEN.items() if route == "dve"}
        dve8 = [inp.tile([_dve_rows(GROUPS[k // 2][1]),
                          2 * _dve_len(GROUPS[k // 2][0])],
                         mybir.dt.int8, name=f"dve8_{k}") for k in range(8)]
        acc_d = inp.tile([128, DVE_F], mybir.dt.float16)
        evac = inp.tile([128, PE_F], mybir.dt.float16)
        psum = psp.tile([128, PE_F], mybir.dt.float32)
        st16 = {g: stgp.tile([128, len(_acts_in(g)) * PE_SLAB_F],
                             mybir.dt.float16, tag="st16", name=f"st16_{g}")
                for g in range(4)}

        # --- DMAs: sync queue = lhst + DVE chunks; pool SWDGE = PE slabs ---
        nc.sync.dma_start(out=lhst[:, :], in_=lhst_d.ap())
        for g in range(4):
            nc.sync.dma_start(out=dve8[2 * g][:, :], in_=dve_d[2 * g].ap())
            nc.gpsimd.dma_start(out=pe8[g][:, :], in_=pe_act_d[g].ap())
            nc.sync.dma_start(out=dve8[2 * g + 1][:, :],
                              in_=dve_d[2 * g + 1].ap())
            for t in range(4 * g, 4 * g + 4):
                if t in WIDEN:
                    dst = x8[t] if WIDEN[t] == "dve" else x16[t]
                    nc.gpsimd.dma_start(out=dst[:, :], in_=pe_x_d[t].ap())

        nc.vector.memset(acc_d[:, :].bitcast(mybir.dt.uint32), 0)

        # Open all 8 psum banks: zero-matmul sets has_written everywhere.
        for i in range(8):
            nc.tensor.matmul(out=psum[:, 512 * i : 512 * i + 512],
                             lhsT=lhst[:, 512:640],
                             rhs=lhst[:, 0:512], start=True, stop=False)

        acc4 = acc_d[:, :].rearrange("p (mu mv q) -> p mu mv q",
                                     mu=8, mv=8, q=32)

        piece_i = [0]

        def rhs_for(t, gR):
            g = t // 4
            if t in WIDEN:
                return x16[t][0:gR, :]
            k = _acts_in(g).index(t)
            return st16[g][0:gR, k * PE_SLAB_F : (k + 1) * PE_SLAB_F]

        # --- per-group emission ---
        for g, (e, f) in enumerate(GROUPS):
            gR = _pe_rows(e, f)
            dR = _dve_rows(f)
            lh = lhst[0:gR, 128 * g : 128 * g + 128]
            # group-level Act widening (one op over the packed act slabs)
            nc.scalar.copy(out=st16[g][0:gR, :], in_=pe8[g][0:gR, :])
            for ai, (a, bb) in enumerate(AB):
                t = 4 * g + ai
                if WIDEN.get(t) == "dve":
                    nc.vector.tensor_copy(out=x16[t][0:gR, :],
                                          in_=x8[t][0:gR, :])
                rhs = rhs_for(t, gR)
                rv = rhs.rearrange("p (iu q) -> p iu q", iu=7, q=448)
                for iu in range(7):
                    m = iu + a
                    nc.tensor.matmul(
                        out=psum[:, m * 512 + bb * 64 : m * 512 + bb * 64 + 448],
                        lhsT=lh, rhs=rv[:, iu],
                        start=False,
                        stop=_BANK_LAST[m] == piece_i[0],
                    )
                    piece_i[0] += 1
                # DVE add (int8 direct, 1 op per term)
                dvv = dve8[t // 2][0:dR,
                                   (t % 2) * _dve_len(e) : (t % 2 + 1) * _dve_len(e)]
                dvv = dvv.rearrange("p (iu iv q) -> p iu iv q",
                                    iu=7, iv=7, q=(8 - e) * 4)
                ov = acc4[0:dR, a : a + 7, bb : bb + 7, 4 * e : 32]
                nc.vector.tensor_add(out=ov, in0=ov, in1=dvv)

        # --- tail ---
        nc.scalar.copy(out=evac[:, 0:2048], in_=psum[:, 0:2048])
        nc.scalar.copy(out=evac[:, 2048:4096], in_=psum[:, 2048:4096])
        nc.sync.dma_start(out=out_dve.ap(), in_=acc_d[:, :])
        nc.scalar.dma_start(out=out_pe.ap(), in_=evac[:, :])
    nc.compile()
    return nc


def kernel(x):
    x = np.ascontiguousarray(np.asarray(x), dtype=np.float32)
    in_maps, scales = _shard(x)
    nc = build_nc()
    from concourse.bass_utils import run_bass_kernel_spmd

    res = run_bass_kernel_spmd(nc, in_maps, core_ids=list(range(8)))
    return _assemble(res.results, scales)


# revision 5
# speedup vs baseline: 1.5960x; 1.0426x over previous
"""DePatchEfficient Trainium2 kernel, v3: PE-major split with on-chip widening.

Overlap-add of 16 polyphase terms. All measured-rate driven (v2 post-mortem):
- DVE int8-source adds run ~1x + 470ns/op (not 2x); Pool tensor ops are
  ~0.2 efficiency (useless); Act casts are 1x + 580ns/op; DVE int8->fp16
  copies run ~2x + 470ns; PE self-loading fp16 matmuls run warm at
  ~N/2.4GHz + 22ns with FWL.

Split (output cells, by (rv, c) inner combo):
- PE owns 4 of 6 combos (rv=0 all c, plus rv=1 c=0): 2/3 of the volume,
  full 8-bank PSUM fp32 acc. Slabs widened to fp16 staging by: Act (bulk,
  one op per group's act-run), DVE tensor_copy (2x), or SWDGE cast-DMA
  (lands fp16 directly). Router 0/1 lhsT absorbs (e,f) shifts + clipping.
  mu stride = 512 = one PSUM bank: every (term, iu) matmul is one
  448-col bank-aligned piece - no bank splitting.
- DVE owns 2 combos (rv=1, c in {1,2}): direct int8 adds into SBUF fp16
  acc, v1-style partition (qt', ws, wt) layout.
- Pool: DMA issue only (SWDGE queue for PE slabs).

PSUM banks are opened by 8 zeroing matmuls (zero lhsT): sets has_written
for every cell so all real matmuls accumulate; no memset needed.

Layouts per core (all cores identical after host-side s/t flips):
- PE psum acc 8 x [128, 512] fp32: partition p = qs*16+qt*2+ru,
  flat free = mu*512 + mv*64 + ws*16 + wt*4 + cmb, cmb in
  {(rv,c)} = [(0,0),(0,1),(0,2),(1,0)]
- PE slab [R, 3136] (R = (8-e)(8-f)*2): row = is*(8-f)*2 + it*2 + ru,
  free = iu*448 + iv*64 + ws*16 + wt*4 + cmb
- DVE acc [128, 2048] fp16: partition = qt'*16 + ws*4 + wt (qt' = 7-qt),
  free = mu*256 + mv*32 + qs*4 + ru*2 + (c-1)
- DVE slab [(8-f)*16, (8-e)*196]: row = qt'*16+ws*4+wt,
  free = iu*(8-e)*28 + iv*(8-e)*4 + is*4 + ru*2 + (c-1)
"""

import os
import sys

import numpy as np

for _p in ("/opt/trn_rl_repo",):
    if os.path.isdir(_p) and _p not in sys.path:
        sys.path.insert(0, _p)

B, U, V, S, T, C = 2, 16, 16, 64, 64, 3
NS, NT, NU, NV = 15, 15, 7, 7

GROUPS = [(1, 1), (1, 0), (0, 1), (0, 0)]      # (e, f) order
AB = [(0, 0), (0, 1), (1, 0), (1, 1)]          # (a, b) within a group
# widening route per term index 0..15: "act" (group fat op), "dve"
# (tensor_copy), "dma" (SWDGE cast-DMA lands fp16 directly)
WIDEN = {2: "dve", 3: "dma", 6: "dve", 11: "dma", 13: "dma", 14: "dve",
         15: "dma"}

PE_SLAB_F = 3136       # (iu7, iv7, ws4, wt4, cmb4)
PE_F = 4096            # psum free: (mu8, mv8, ws4, wt4, cmb4)
DVE_F = 2048


def _pe_rows(e, f):
    return (8 - e) * (8 - f) * 2


def _dve_rows(f):
    return (8 - f) * 16


def _dve_len(e):
    return 49 * (8 - e) * 4


def _acts_in(g):
    """Term indices in group g widened by Act (contiguous run, packed)."""
    return [4 * g + i for i in range(4) if WIDEN.get(4 * g + i) is None]


def _lhst_mats():
    """4 router matrices + 1 zero matrix, [128, 128] fp16 each."""
    mats = np.zeros((5, 128, 128), np.float16)
    for gi, (e, f) in enumerate(GROUPS):
        for is_ in range(8 - e):
            for it in range(8 - f):
                for ru in range(2):
                    r = is_ * (8 - f) * 2 + it * 2 + ru
                    p = (is_ + e) * 16 + (it + f) * 2 + ru
                    mats[gi, r, p] = 1.0
    return mats


_LHST = _lhst_mats()

# emission-ordered matmul pieces (term, iu, bank, b): groups 0-2
# term-major; group 3 bank-major so banks complete progressively and the
# per-bank Pool evacuation + out-DMA pipeline behind the matmul stream.
_PIECES = []
for _g in range(3):
    for _ai, (_a, _b) in enumerate(AB):
        for _iu in range(7):
            _PIECES.append((4 * _g + _ai, _iu, _iu + _a, _b))
for _m in range(8):
    for _ai, (_a, _b) in enumerate(AB):
        _iu = _m - _a
        if 0 <= _iu < 7:
            _PIECES.append((12 + _ai, _iu, _m, _b))
_BANK_LAST = {}
for _i, (_t, _iu, _m, _b) in enumerate(_PIECES):
    _BANK_LAST[_m] = _i


def _shard(x):
    """Full input -> per-core in_maps + scales.

    Per core buffers:
    - lhst [128, 640] fp16
    - pe_act{g} [R, len(_acts_in(g))*3136] int8  (group-major packed)
    - pe_x{t} [R, 3136] int8 (dve-widened) or fp16 (dma-widened)
    - dve_c{k} [R', 2*dve_len] int8, k = 0..7 (2 terms each)
    """
    x9 = np.ascontiguousarray(x).reshape(B, NS, NT, NU, NV, 4, 4, 8, 8, C)
    in_maps, scales = [], []
    for core in range(8):
        b, sh, th = core // 4, (core // 2) % 2, core % 2
        xc = x9[b, 7 * sh : 7 * sh + 8, 7 * th : 7 * th + 8]
        if sh:
            xc = xc[::-1, :, :, :, :, :, ::-1]
        if th:
            xc = xc[:, ::-1, :, :, :, :, :, ::-1]
        s = float(np.abs(xc).max()) / 127.0
        xq = np.clip(np.rint(xc * (1.0 / s)), -127, 127).astype(np.int8)
        # (is, it, iu, iv, a, ru, b, rv, e, ws, f, wt, c)
        xq = xq.reshape(8, 8, NU, NV, 2, 2, 2, 2, 2, 4, 2, 4, C)
        scales.append(s)
        m = {"lhst": _LHST.transpose(1, 0, 2).reshape(128, 640).copy()}

        pe_slabs, dve_bufs = [], []
        for (e, f) in GROUPS:
            for (a, bb) in AB:
                # PE slab: (is, it, ru | iu, iv, ws, wt, cmb4)
                # cmb: (rv=0, c=0..2) then (rv=1, c=0)
                r0 = xq[: 8 - e, : 8 - f, :, :, a, :, bb, 0, e, :, f, :, :]
                r1 = xq[: 8 - e, : 8 - f, :, :, a, :, bb, 1, e, :, f, :, 0:1]
                sl = np.concatenate([r0, r1], axis=7)  # (...ws, wt, cmb4)
                sl = sl.transpose(0, 1, 4, 2, 3, 5, 6, 7)
                pe_slabs.append(
                    np.ascontiguousarray(sl.reshape(_pe_rows(e, f), PE_SLAB_F)))
                # DVE slab: rv=1, c in {1,2} -> (it', ws, wt | iu, iv, is, ru, c)
                g2 = xq[: 8 - e, : 8 - f, :, :, a, :, bb, 1, e, :, f, :, 1:]
                g2 = g2[:, ::-1].transpose(1, 5, 6, 2, 3, 0, 4, 7)
                dve_bufs.append(
                    np.ascontiguousarray(g2.reshape(_dve_rows(f), _dve_len(e))))

        for g in range(4):
            acts = _acts_in(g)
            m[f"pe_act{g}"] = np.ascontiguousarray(
                np.concatenate([pe_slabs[t] for t in acts], axis=1))
        for t, route in WIDEN.items():
            sl = pe_slabs[t]
            m[f"pe_x{t}"] = sl if route == "dve" else sl.astype(np.float16)
        for k in range(8):
            m[f"dve_c{k}"] = np.ascontiguousarray(
                np.concatenate(dve_bufs[2 * k : 2 * k + 2], axis=1))
        in_maps.append(m)
    return in_maps, scales


def _count_map():
    cu = np.array([1, 2, 2, 2, 2, 2, 2, 1], np.float32)[np.arange(U) // 2]
    cs = np.array([1, 2, 2, 2, 2, 2, 2, 2], np.float32)[np.arange(32) // 4]
    cnt = (cu[:, None, None, None] * cu[None, :, None, None]
           * cs[None, None, :, None] * cs[None, None, None, :])
    return cnt[..., None]  # (U, V, 32, 32, 1)


_CNT = _count_map()


def _assemble(core_outs, scales):
    """Per-core {out_pe, out_dve} -> full (B, U, V, S, T, C) fp32."""
    full = np.empty((B, U, V, S, T, C), np.float32)
    for core in range(8):
        b, sh, th = core // 4, (core // 2) % 2, core % 2
        vol = np.zeros((U, V, 32, 32, C), np.float32)
        pe = np.asarray(core_outs[core]["out_pe"], np.float32)
        # (qs, qt, ru | mu, mv, ws, wt, cmb)
        pe = pe.reshape(8, 8, 2, 8, 8, 4, 4, 4)
        # -> (mu, ru, mv, qs, ws, qt, wt, cmb)
        pe = pe.transpose(3, 2, 4, 0, 5, 1, 6, 7).reshape(U, 8, 32, 32, 4)
        vol[:, 0::2, :, :, :] = pe[..., 0:3]        # rv=0, c=0..2
        vol[:, 1::2, :, :, 0] = pe[..., 3]          # rv=1, c=0
        dv = np.asarray(core_outs[core]["out_dve"], np.float32)
        # (qt', ws, wt | mu, mv, qs, ru, c2)
        dv = dv.reshape(8, 4, 4, 8, 8, 8, 2, 2)
        dv = dv[::-1].transpose(3, 6, 4, 5, 1, 0, 2, 7)
        vol[:, 1::2, :, :, 1:] = dv.reshape(U, 8, 32, 32, 2)

        vol *= scales[core] / _CNT
        if sh:
            vol = vol[:, :, ::-1]
        if th:
            vol = vol[:, :, :, ::-1]
        full[b, :, :, 32 * sh : 32 * sh + 32, 32 * th : 32 * th + 32, :] = vol
    return full


def build_nc():
    import concourse.bacc as bacc
    import concourse.mybir as mybir
    from concourse.tile import TileContext

    nc = bacc.Bacc("TRN2", target_bir_lowering=False, debug=False)

    lhst_d = nc.dram_tensor("lhst", [128, 640], mybir.dt.float16,
                            kind="ExternalInput")
    pe_act_d, pe_x_d, dve_d = {}, {}, []
    for g, (e, f) in enumerate(GROUPS):
        pe_act_d[g] = nc.dram_tensor(
            f"pe_act{g}", [_pe_rows(e, f), len(_acts_in(g)) * PE_SLAB_F],
            mybir.dt.int8, kind="ExternalInput")
    for t, route in WIDEN.items():
        e, f = GROUPS[t // 4]
        pe_x_d[t] = nc.dram_tensor(
            f"pe_x{t}", [_pe_rows(e, f), PE_SLAB_F],
            mybir.dt.int8 if route == "dve" else mybir.dt.float16,
            kind="ExternalInput")
    for k in range(8):
        e, f = GROUPS[k // 2]
        dve_d.append(nc.dram_tensor(
            f"dve_c{k}", [_dve_rows(f), 2 * _dve_len(e)], mybir.dt.int8,
            kind="ExternalInput"))
    out_pe = nc.dram_tensor("out_pe", [128, PE_F], mybir.dt.float16,
                            kind="ExternalOutput")
    out_dve = nc.dram_tensor("out_dve", [128, DVE_F], mybir.dt.float16,
                             kind="ExternalOutput")

    with (
        TileContext(nc) as tc,
        tc.tile_pool(name="inp", bufs=1) as inp,
        tc.tile_pool(name="stgp", bufs=2) as stgp,
        tc.tile_pool(name="psp", bufs=1, space="PSUM") as psp,
    ):
        lhst = inp.tile([128, 640], mybir.dt.float16)
        pe8 = {g: inp.tile([_pe_rows(*GROUPS[g]), len(_acts_in(g)) * PE_SLAB_F],
                           mybir.dt.int8, name=f"pe8_{g}") for g in range(4)}
        x16 = {t: inp.tile([_pe_rows(*GROUPS[t // 4]), PE_SLAB_F],
                           mybir.dt.float16, name=f"x16_{t}")
               for t in WIDEN}
        x8 = {t: inp.tile([_pe_rows(*GROUPS[t // 4]), PE_SLAB_F],
                          mybir.dt.int8, name=f"x8_{t}")
              for t, route in WIDEN.items() if route == "dve"}
        dve8 = [inp.tile([_dve_rows(GROUPS[k // 2][1]),
                          2 * _dve_len(GROUPS[k // 2][0])],
                         mybir.dt.int8, name=f"dve8_{k}") for k in range(8)]
        acc_d = inp.tile([128, DVE_F], mybir.dt.float16)
        evac = inp.tile([128, PE_F], mybir.dt.float16)
        banks = [psp.tile([128, 512], mybir.dt.float32, name=f"bank{i}")
                 for i in range(8)]
        st16 = {g: stgp.tile([128, len(_acts_in(g)) * PE_SLAB_F],
                             mybir.dt.float16, tag="st16", name=f"st16_{g}")
                for g in range(4)}

        # --- DMAs: sync queue = lhst + DVE chunks; pool SWDGE = PE slabs ---
        nc.sync.dma_start(out=lhst[:, :], in_=lhst_d.ap())
        for g in range(4):
            nc.sync.dma_start(out=dve8[2 * g][:, :], in_=dve_d[2 * g].ap())
            nc.gpsimd.dma_start(out=pe8[g][:, :], in_=pe_act_d[g].ap())
            nc.sync.dma_start(out=dve8[2 * g + 1][:, :],
                              in_=dve_d[2 * g + 1].ap())
            for t in range(4 * g, 4 * g + 4):
                if t in WIDEN:
                    dst = x8[t] if WIDEN[t] == "dve" else x16[t]
                    nc.gpsimd.dma_start(out=dst[:, :], in_=pe_x_d[t].ap())

        nc.vector.memset(acc_d[:, :].bitcast(mybir.dt.uint32), 0)

        # Open all 8 psum banks: zero-matmul sets has_written everywhere.
        for i in range(8):
            nc.tensor.matmul(out=banks[i][:, :], lhsT=lhst[:, 512:640],
                             rhs=lhst[:, 0:512], start=True, stop=False)

        acc4 = acc_d[:, :].rearrange("p (mu mv q) -> p mu mv q",
                                     mu=8, mv=8, q=32)

        def rhs_for(t):
            gR = _pe_rows(*GROUPS[t // 4])
            if t in WIDEN:
                return x16[t][0:gR, :]
            g = t // 4
            k = _acts_in(g).index(t)
            return st16[g][0:gR, k * PE_SLAB_F : (k + 1) * PE_SLAB_F]

        def emit_cast(t):
            g = t // 4
            gR = _pe_rows(*GROUPS[g])
            if WIDEN.get(t) == "dve":
                nc.vector.tensor_copy(out=x16[t][0:gR, :], in_=x8[t][0:gR, :])
            elif t not in WIDEN:
                k = _acts_in(g).index(t)
                sl = slice(k * PE_SLAB_F, (k + 1) * PE_SLAB_F)
                nc.scalar.copy(out=st16[g][0:gR, sl], in_=pe8[g][0:gR, sl])

        def emit_add(t):
            e, f = GROUPS[t // 4]
            a, bb = AB[t % 4]
            dR = _dve_rows(f)
            dvv = dve8[t // 2][0:dR,
                               (t % 2) * _dve_len(e) : (t % 2 + 1) * _dve_len(e)]
            dvv = dvv.rearrange("p (iu iv q) -> p iu iv q",
                                iu=7, iv=7, q=(8 - e) * 4)
            ov = acc4[0:dR, a : a + 7, bb : bb + 7, 4 * e : 32]
            nc.vector.tensor_add(out=ov, in0=ov, in1=dvv)

        rvs = {}

        def emit_mm(pi):
            t, iu, m, bb = _PIECES[pi]
            if t not in rvs:
                rvs[t] = rhs_for(t).rearrange("p (iu q) -> p iu q", iu=7, q=448)
            g = t // 4
            gR = _pe_rows(*GROUPS[g])
            nc.tensor.matmul(
                out=banks[m][:, bb * 64 : bb * 64 + 448],
                lhsT=lhst[0:gR, 128 * g : 128 * g + 128],
                rhs=rvs[t][:, iu],
                start=False, stop=_BANK_LAST[m] == pi)
            if _BANK_LAST[m] == pi:
                nc.scalar.copy(out=evac[:, 512 * m : 512 * m + 512],
                               in_=banks[m][:, :])

        # groups 0-2: term-major
        pi = 0
        for g in range(3):
            for t in range(4 * g, 4 * g + 4):
                emit_cast(t)
                for _ in range(7):
                    emit_mm(pi)
                    pi += 1
                emit_add(t)
        # group 3: casts/copies first, then bank-major matmuls (+ evacs),
        # adds interleaved
        for t in range(12, 16):
            emit_cast(t)
        adds_left = list(range(12, 16))
        while pi < len(_PIECES):
            emit_mm(pi)
            pi += 1
            if adds_left and pi % 4 == 0:
                emit_add(adds_left.pop(0))
        for t in adds_left:
            emit_add(t)

        # --- outputs ---
        nc.sync.dma_start(out=out_dve.ap(), in_=acc_d[:, :])
        nc.sync.dma_start(out=out_pe.ap()[:, 0:2048], in_=evac[:, 0:2048])
        nc.sync.dma_start(out=out_pe.ap()[:, 2048:4096], in_=evac[:, 2048:4096])
    nc.compile()
    return nc


def kernel(x):
    x = np.ascontiguousarray(np.asarray(x), dtype=np.float32)
    in_maps, scales = _shard(x)
    nc = build_nc()
    from concourse.bass_utils import run_bass_kernel_spmd

    res = run_bass_kernel_spmd(nc, in_maps, core_ids=list(range(8)))
    return _assemble(res.results, scales)
